# revision 40
# baseline (speedup 1.0000x reference)
"""ChromaticityShiftSRGB Trainium2 kernel (8 NeuronCores, SPMD batch-sharded).

v3 (566us v1 -> ~516us): software-pipelined emission + chunked psums.
  - emission order per supertile k: DMA-in(k), per-stripe Ln/Exp/S2L(k),
    out-stage(k-1) [4 MMs + Ln/Exp + OSEL per (stripe, 512-chunk) through
    a 2-deep [126,512] pf ring], DMA-out(k-1), in-MMs + pixel chain(k).
    This puts in-Ln/Exp(k) AHEAD of out-Ln/Exp(k-1) in the scalar queue
    (S2L(k) starts ~4us earlier) and OSEL(k-1) fills the vector engine's
    wait on in-MM(k).
  - input psums are [126,512] ring-2 (3 tags x 1 bank x 2 + pf 2 = 8
    banks): in-MM(k, c1) overlaps chain(k, c0), and in-MM(k+1) never
    waits on chain(k).
  - per-stripe input Ln/Exp/S2L (6 ops each) cuts the input head latency.
  - weights inlined as fp16 and DMA'd directly (no staging CAST on DVE).
  - the fp16 hi/lo DATA split and the y/D weight-split passes are load
    bearing: 11-bit-only y/D flips the sign of v'2 for ~11/16.7M pixels
    sitting on the chromaticity pole, where the reference output is a
    genuine 0<->1 step (error 1.0). Numpy-model absmax (1.1e-2) does NOT
    catch this - the flips come from sub-model-precision perturbations
    (ACT table error etc.) on knife-edge pixels.
  Measured dead ends (kept out): GPSIMD pixel offload (+35us, serializes
  the chain), ACT psum->SBUF copies, fp32r matmuls, fp16-cast input
  (1.85e-2), hi-only data stream (pole flips), pass-major in-MM order
  (hammers one psum bank with 9 consecutive accumulating MMs),
  interleaving OSEL(k-1) between S2L stripes (stalls vector on unready
  OSELs), LDWEIGHTS pairing (LDW fully hidden behind MM at steady state).
  NOTE: HW run-to-run variance is up to ~100us (DVFS/throttle state);
  compare configs with min-of-2 runs.

Pipeline per pixel:
  lin = srgb_to_linear(x); y/D/nu = stripe-sel matmuls (split-fp16 y,D)
  L = select(y<=d3, kappa*y, 116*y^(1/3)-16); S = D/L
  t1 = y/(y - (5/117)S); nu2 = nu + (5/13)S
  A = clamp(t1)*nu2; Kq = clamp(t1)*D
  lin2 = G @ (A, Kq, y);  out = clamp01(linear_to_srgb(lin2))
"""
import sys
import os
import numpy as np

for _p in ("/opt/trn_rl_repo", "/root/.axon_site/_ro/trn_rl_repo"):
    if os.path.isdir(_p) and _p not in sys.path:
        sys.path.insert(0, _p)
        break

import ml_dtypes
import contextlib

import concourse.bass as bass
import concourse.bacc as bacc
import concourse.tile as tile
from concourse import mybir
from concourse.bass_utils import run_bass_kernel_spmd
from concourse import dve_ops
from concourse.dve_spec import (
    Spec, Src0, Src1, C0, C1, C2, AluOp, Bin, select, lower, _has_src1,
    Zero, One, maxx, minn,
)
from concourse.dve_uop import DveOpSpec

F32 = mybir.dt.float32
BF16 = mybir.dt.bfloat16
F16 = mybir.dt.float16
AF = mybir.ActivationFunctionType

N_CORES = 8
IMG_PER_CORE = 2
H = 1024
WID = 1024
GB = 42          # pixel-group block (3*GB=126 partitions)
N_STRIPES = 3    # stripes per supertile
SUP_ROWS = GB * N_STRIPES          # 126 rows per supertile
N_SUP = H // SUP_ROWS              # 8 supertiles
TAIL_ROWS = H - N_SUP * SUP_ROWS   # 16
CHUNK = 512
N_CHUNK = WID // CHUNK             # 2

SPLIT_MODE = os.environ.get("CHR_SPLIT_MODE", "dve")

# Pin Ln+Exp to the shared "natural_log_exp_and_others" ACT table set.
from concourse import hw_specs as _hw_specs

_orig_gat = _hw_specs.get_activation_tables


def _gat_pinned(arch):
    t = _orig_gat(arch)
    for name, funcs in t.items():
        if name != "natural_log_exp_and_others":
            funcs.discard(AF.Ln)
            funcs.discard(AF.Exp)
    return t


bacc.get_activation_tables = _gat_pinned

# ---------------- host-side constants (mirror reference.py) ----------------
_W = np.array([[0.4124564, 0.3575761, 0.1804375],
               [0.2126728, 0.7151522, 0.0721750],
               [0.0193339, 0.1191920, 0.9503041]], dtype=np.float32)
_W_INV = np.linalg.inv(_W.T).astype(np.float32)
_M = _W_INV.T
_w1, _w2, _w3 = _W[0], _W[1], _W[2]
_wD = (_w1 + 15.0 * _w2 + 3.0 * _w3).astype(np.float32)
_N = np.array([[0.25, 0.0, 0.0],
               [0.0, 0.0, 1.0],
               [-1.0 / 12.0, 1.0 / 3.0, -5.0]], dtype=np.float64)
_G = (_M.astype(np.float64) @ _N).astype(np.float32)

DELTA3 = float((6.0 / 29.0) ** 3)
KAPPA = float((29.0 / 3.0) ** 3)
C5_13 = float(5.0 / 13.0)
C5_117 = float(5.0 / 117.0)
LN116 = float(np.log(116.0))
LN1055 = float(np.log(1.055))
MASK_F = 1.9990234375
T1_CLAMP = 1000.0

# ---------------- custom DVE ops ----------------

def _register_op(name, spec, subdim=False):
    for op in dve_ops.OPS:
        if op.name == name:
            return op
    shas = {}
    for ver in ("v3", "v4"):
        uops = lower(spec, ver=ver)
        tmp = DveOpSpec(name=name, opcode=0, uops=uops, rd1_en=_has_src1(spec))
        shas[ver] = tmp.sha(ver)
    op = dve_ops.DveOp(name, spec, subdim, uops_sha=shas)
    dve_ops.OPS.append(op)
    dve_ops.CUSTOM_DVE_SPECS[name] = spec
    dve_ops._SUB_OPCODE_FOR_NAME[name] = (
        dve_ops._CUSTOM_DVE_ROW_BASE + len(dve_ops.OPS) - 1
    )
    assert dve_ops._SUB_OPCODE_FOR_NAME[name] < 0x20
    return op


def _np_mask(c1):
    m = np.ascontiguousarray(np.asarray(c1, np.float32)).view(np.uint32)
    return m if m.ndim == 0 else m.reshape(-1, 1)


def _ref_blend(in0, in1, c0, c2):
    return np.where(in0 <= np.float32(c0),
                    in0.astype(np.float32) * np.float32(c2),
                    in1.astype(np.float32)).astype(np.float32)


def _ref_s2l_hi(in0, in1, c0, c1, c2):
    b = _ref_blend(in0, in1, c0, c2)
    return (b.view(np.uint32) & _np_mask(c1)).view(np.float32)


def _ref_s2l_lo(in0, in1, c0, c1, c2):
    b = _ref_blend(in0, in1, c0, c2)
    hi = (b.view(np.uint32) & _np_mask(c1)).view(np.float32)
    return (b - hi).astype(np.float32)


def _ref_lsel(in0, in1, c0, c1, c2):
    return np.where(in0 <= np.float32(c0),
                    in0.astype(np.float32) * np.float32(c1),
                    in1.astype(np.float32) - np.float32(c2)).astype(np.float32)


def _ref_blend3(in0, in1, c0, c1, c2):
    return _ref_blend(in0, in1, c0, c2)


def _ref_osel(in0, in1, c0, c1, c2):
    v = np.where(in0 <= np.float32(c0),
                 in0.astype(np.float32) * np.float32(c1),
                 in1.astype(np.float32) - np.float32(c2)).astype(np.float32)
    return np.clip(v, np.float32(0.0), np.float32(1.0)).astype(np.float32)


_blend = select(Src0 <= C0, Src0 * C2, Src1)
OP_S2L_HI = _register_op(
    "ANT_CHR_S2L_HI",
    Spec(body=Bin(AluOp.BITWISE_AND, _blend, C1), reference=_ref_s2l_hi))
OP_S2L_LO = _register_op(
    "ANT_CHR_S2L_LO",
    Spec(body=_blend - Bin(AluOp.BITWISE_AND, _blend, C1),
         reference=_ref_s2l_lo))
OP_BLEND = _register_op(
    "ANT_CHR_BLEND",
    Spec(body=select(Src0 <= C0, Src0 * C2, Src1), reference=_ref_blend3))
OP_LSEL = _register_op(
    "ANT_CHR_LSEL",
    Spec(body=select(Src0 <= C0, Src0 * C1, Src1 - C2), reference=_ref_lsel))
# clamp01 folded into the branches; zero synthesized as t-t to stay
# within the DVE's 6 carry lanes (Src0, Src1, C0, C1, C2, One).
_ot = Src0 * C1
_olow = maxx(_ot, Bin(AluOp.SUBTRACT, _ot, _ot))
_ohi = minn(Src1 - C2, One)
OP_OSEL = _register_op(
    "ANT_CHR_OSEL",
    Spec(body=select(Src0 <= C0, _olow, _ohi), reference=_ref_osel))

OP_CLAMP_MUL = _register_op(
    "ANT_CHR_CLAMP_MUL",
    Spec(body=minn(maxx(Src0, C1), C0) * Src1,
         reference=lambda in0, in1, c0, c1, c2:
         (np.clip(in0.astype(np.float32), np.float32(c1), np.float32(c0))
          * in1.astype(np.float32)).astype(np.float32)))

# t1c = clamp(in0*in1, [c1, c0]) — fuses t1 = y*rdv with the +-1000 clamp.
OP_TCLAMP = _register_op(
    "ANT_CHR_TCLAMP",
    Spec(body=minn(maxx(Src0 * Src1, C1), C0),
         reference=lambda in0, in1, c0, c1, c2:
         np.clip(in0.astype(np.float32) * in1.astype(np.float32),
                 np.float32(c1), np.float32(c0)).astype(np.float32)))


# ---------------- weight construction ----------------

def _fp16_grid(a):
    return np.asarray(a, np.float32).astype(np.float16).astype(np.float32)


def _build_weights(g, n_str):
    """Weight matrices for block size g with n_str stripes.
    cp = 3g channel partitions; pp = g*n_str pixel partitions."""
    cp, pp = 3 * g, g * n_str
    out = {}

    def sel_combo(w):
        mats = []
        for s in range(n_str):
            m = np.zeros((cp, pp), np.float32)
            for c in range(3):
                for j in range(g):
                    m[g * c + j, g * s + j] = w[c]
            mats.append(m)
        return mats

    def split(mats):
        his, los = [], []
        for m in mats:
            hi = _fp16_grid(m)
            lo = _fp16_grid(m - hi)
            his.append(hi)
            los.append(lo)
        return his, los

    out["y_hi"], out["y_lo"] = split(sel_combo(_w2))
    out["d_hi"], out["d_lo"] = split(sel_combo(_wD))
    out["p4"] = [_fp16_grid(m) for m in sel_combo(4.0 * _w1)]

    def sel_back(col):
        mats = []
        for s in range(n_str):
            m = np.zeros((pp, cp), np.float32)
            for c2 in range(3):
                for j in range(g):
                    m[g * s + j, g * c2 + j] = _G[c2, col]
            mats.append(m)
        return mats

    out["ga"] = [_fp16_grid(m) for m in sel_back(0)]
    out["gk"] = [_fp16_grid(m) for m in sel_back(1)]
    gy = np.zeros((cp, cp), np.float32)
    for c in range(3):
        for c2 in range(3):
            for j in range(g):
                gy[g * c + j, g * c2 + j] = _w2[c] * _G[c2, 2]
    out["gy"] = [_fp16_grid(gy)]
    return out


# ---------------- program builder ----------------

def build_program(n_img=IMG_PER_CORE, h=H, wid=WID):
    n_sup = h // SUP_ROWS
    tail_rows = h - n_sup * SUP_ROWS
    n_chunk = wid // CHUNK

    nc = bacc.Bacc("TRN2", target_bir_lowering=False, debug=False)
    im_d = nc.declare_dram_parameter("im", [n_img, 3, h, wid], F32, isOutput=False)
    out_d = nc.declare_dram_parameter("out", [n_img, 3, h, wid], F32, isOutput=True)

    wmain = _build_weights(GB, N_STRIPES)
    # Merged tail: both images' leftover rows in ONE stage (g = n_img*16
    # = 32 rows, cp = 96). DVE/ACT op cost is column-bound, so two
    # 16-row tails cost twice what one 32-row tail does.
    wtail = _build_weights(n_img * tail_rows, 1) if tail_rows else None
    assert 3 * n_img * tail_rows <= 128

    def to_dram(wdict, pfx):
        d = {}
        for k, lst in wdict.items():
            for s, m in enumerate(lst):
                d[(k, s)] = nc.inline_tensor(
                    np.ascontiguousarray(m.astype(np.float16)),
                    name=f"w_{pfx}_{k}_{s}")
        return d

    wmain_dram = to_dram(wmain, "m")
    wtail_dram = to_dram(wtail, "t") if wtail else None

    with tile.TileContext(nc) as tc:
        with contextlib.ExitStack() as ctx:
            wp = ctx.enter_context(tc.tile_pool(name="wp", bufs=1))
            xio = ctx.enter_context(tc.tile_pool(name="xio", bufs=3))
            chan = ctx.enter_context(tc.tile_pool(name="chan", bufs=3))
            pix = ctx.enter_context(tc.tile_pool(name="pix", bufs=3))
            ost = ctx.enter_context(tc.tile_pool(name="ost", bufs=2))
            pmm = ctx.enter_context(tc.tile_pool(name="pmm", bufs=2, space="PSUM"))
            pf = ctx.enter_context(tc.tile_pool(name="pf", bufs=3, space="PSUM"))

            # ---- load weights / consts ----
            def load_weights(drams, pfx):
                tiles = {}
                for key, dh in drams.items():
                    shp = list(dh.shape)
                    t = wp.tile(shp, F16, tag=f"w_{pfx}_{key[0]}_{key[1]}")
                    nc.sync.dma_start(out=t[:], in_=dh[:])
                    tiles[key] = t
                return tiles

            # (load_weights is called after the first input DMA below so
            # the first supertile's image rows lead the sync queue)

            def bias(v, name):
                t = wp.tile([128, 1], F32, tag=f"bias_{name}")
                nc.vector.memset(t[:], float(v))
                return t

            b_s2l = bias(0.055 / 1.055, "s2l")
            b_ln116 = bias(LN116, "ln116")
            b_ln1055 = bias(LN1055, "ln1055")
            b_zero = bias(0.0, "zero")

            def input_dma(n, r0, g, n_str, wts):
                """DMA-in only; returns supertile context."""
                cp = 3 * g          # channel-block partitions
                pp = g * n_str      # pixel-block partitions

                x = xio.tile([cp, n_str, wid], F32, tag="x")
                if n < 0:     # merged tail: g rows = n_img * tail_rows
                    for c in range(3):
                        for nn in range(n_img):
                            nc.sync.dma_start(
                                out=x[g * c + tail_rows * nn:
                                      g * c + tail_rows * (nn + 1), 0, :],
                                in_=im_d[nn, c, r0:r0 + tail_rows, :])
                else:
                    for c in range(3):
                        nc.sync.dma_start(
                            out=x[g * c:g * c + g, :, :],
                            in_=im_d[n, c, r0:r0 + g * n_str, :]
                            .rearrange("(s j) w -> j s w", j=g))

                q = chan.tile([cp, n_str, wid], F32, tag="qq")
                hi = chan.tile([cp, n_str, wid], F16, tag="hi")
                lo = chan.tile([cp, n_str, wid], F16, tag="lo")
                return dict(n=n, r0=r0, g=g, n_str=n_str, wts=wts,
                            cp=cp, pp=pp, x=x, q=q, hi=hi, lo=lo)

            def input_stripe(st, s):
                """Ln/Exp + S2L split for one stripe."""
                cp, x, q = st["cp"], st["x"], st["q"]
                hi, lo = st["hi"], st["lo"]
                nc.scalar.activation(q[:, s, :], x[:, s, :], AF.Ln,
                                     bias=b_s2l[0:cp],
                                     scale=float(1 / 1.055))
                nc.scalar.activation(q[:, s, :], q[:, s, :], AF.Exp,
                                     bias=b_zero[0:cp], scale=2.4)
                nc.vector._custom_dve(OP_S2L_HI, out=hi[:, s, :],
                                      in0=x[:, s, :],
                                      in1=q[:, s, :],
                                      s0=0.04045, s1=MASK_F,
                                      imm2=float(1 / 12.92))
                nc.vector._custom_dve(OP_S2L_LO, out=lo[:, s, :],
                                      in0=x[:, s, :],
                                      in1=q[:, s, :],
                                      s0=0.04045, s1=MASK_F,
                                      imm2=float(1 / 12.92))

            def mm_chain(st):
                """Input matmuls + per-chunk pixel chain -> A/Kq (fp16)."""
                pp, n_str, wts = st["pp"], st["n_str"], st["wts"]
                hi, lo = st["hi"], st["lo"]

                A_full = ost.tile([pp, wid], F16, tag="Afull")
                Kq_full = ost.tile([pp, wid], F16, tag="Kqfull")
                for hch in range(n_chunk):
                    cs = slice(CHUNK * hch, CHUNK * (hch + 1))
                    psum_y = pmm.tile([pp, CHUNK], F32, tag="psy")
                    psum_d = pmm.tile([pp, CHUNK], F32, tag="psd")
                    psum_nu = pmm.tile([pp, CHUNK], F32, tag="psnu", bufs=1)
                    for s in range(n_str):
                        first = s == 0
                        last = s == n_str - 1
                        nc.tensor.matmul(psum_y[:], lhsT=wts[("y_hi", s)][:],
                                         rhs=hi[:, s, cs], start=first, stop=False)
                        nc.tensor.matmul(psum_y[:], lhsT=wts[("y_hi", s)][:],
                                         rhs=lo[:, s, cs], start=False, stop=False)
                        nc.tensor.matmul(psum_y[:], lhsT=wts[("y_lo", s)][:],
                                         rhs=hi[:, s, cs], start=False, stop=last)
                        nc.tensor.matmul(psum_d[:], lhsT=wts[("d_hi", s)][:],
                                         rhs=hi[:, s, cs], start=first, stop=False)
                        nc.tensor.matmul(psum_d[:], lhsT=wts[("d_hi", s)][:],
                                         rhs=lo[:, s, cs], start=False, stop=False)
                        nc.tensor.matmul(psum_d[:], lhsT=wts[("d_lo", s)][:],
                                         rhs=hi[:, s, cs], start=False, stop=last)
                        nc.tensor.matmul(psum_nu[:], lhsT=wts[("p4", s)][:],
                                         rhs=hi[:, s, cs], start=first, stop=last)

                    lny = pix.tile([pp, CHUNK], F32, tag="lnyL")
                    nc.scalar.activation(lny[:], psum_y[:], AF.Ln,
                                         bias=b_zero[0:pp], scale=1.0)
                    e3 = pix.tile([pp, CHUNK], F32, tag="e3rL")
                    nc.scalar.activation(e3[:], lny[:], AF.Exp,
                                         bias=b_ln116[0:pp], scale=float(1 / 3))
                    L = pix.tile([pp, CHUNK], F32, tag="lnyL")
                    nc.vector._custom_dve(OP_LSEL, out=L[:], in0=psum_y[:],
                                          in1=e3[:],
                                          s0=DELTA3, s1=KAPPA, imm2=16.0)
                    rL = pix.tile([pp, CHUNK], F32, tag="e3rL")
                    nc.vector.reciprocal_approx_fast(out=rL[:], in_=L[:])
                    S = pix.tile([pp, CHUNK], F32, tag="SKq")
                    nc.vector.tensor_mul(S[:], rL[:], psum_d[:])
                    dv = pix.tile([pp, CHUNK], F32, tag="dvnu")
                    nc.vector.scalar_tensor_tensor(dv[:], in0=S[:],
                                                   scalar=-C5_117, in1=psum_y[:],
                                                   op0=mybir.AluOpType.mult,
                                                   op1=mybir.AluOpType.add)
                    rdv = pix.tile([pp, CHUNK], F32, tag="rdvt1c")
                    nc.vector.reciprocal_approx_fast(out=rdv[:], in_=dv[:])
                    t1 = pix.tile([pp, CHUNK], F32, tag="t1A")
                    nc.vector.tensor_mul(t1[:], rdv[:], psum_y[:])
                    nu2 = pix.tile([pp, CHUNK], F32, tag="dvnu")
                    nc.vector.scalar_tensor_tensor(nu2[:], in0=S[:],
                                                   scalar=C5_13, in1=psum_nu[:],
                                                   op0=mybir.AluOpType.mult,
                                                   op1=mybir.AluOpType.add)
                    nc.vector._custom_dve(OP_CLAMP_MUL, out=A_full[:, cs],
                                          in0=t1[:],
                                          in1=nu2[:], s0=T1_CLAMP, s1=-T1_CLAMP)
                    nc.vector._custom_dve(OP_CLAMP_MUL, out=Kq_full[:, cs],
                                          in0=t1[:],
                                          in1=psum_d[:], s0=T1_CLAMP,
                                          s1=-T1_CLAMP)
                st["A"], st["Kq"] = A_full, Kq_full

            def out_group(st, s, hch):
                """One (stripe, chunk) out group: 4 MMs + Ln/Exp + OSEL."""
                cp, wts = st["cp"], st["wts"]
                hi, lo = st["hi"], st["lo"]
                A_full, Kq_full = st["A"], st["Kq"]
                outb = st["outb"]
                cs = slice(CHUNK * hch, CHUNK * (hch + 1))
                psum_f = pf.tile([cp, CHUNK], F32, tag="psf")
                nc.tensor.matmul(psum_f[:], lhsT=wts[("ga", s)][:],
                                 rhs=A_full[:, cs], start=True, stop=False)
                nc.tensor.matmul(psum_f[:], lhsT=wts[("gk", s)][:],
                                 rhs=Kq_full[:, cs], start=False, stop=False)
                nc.tensor.matmul(psum_f[:], lhsT=wts[("gy", 0)][:],
                                 rhs=hi[:, s, cs], start=False, stop=False)
                nc.tensor.matmul(psum_f[:], lhsT=wts[("gy", 0)][:],
                                 rhs=lo[:, s, cs], start=False, stop=True)

                q2 = ost.tile([cp, CHUNK], F32, tag="q2", bufs=3)
                nc.scalar.activation(q2[:], psum_f[:], AF.Ln,
                                     bias=b_zero[0:cp], scale=1.0)
                E = q2
                nc.scalar.activation(E[:], q2[:], AF.Exp,
                                     bias=b_ln1055[0:cp],
                                     scale=float(1 / 2.4))
                nc.vector._custom_dve(OP_OSEL, out=outb[:, s, cs],
                                      in0=psum_f[:],
                                      in1=E[:], s0=0.0031308, s1=12.92,
                                      imm2=0.055)

            def out_dma(st):
                n, r0, g, n_str = st["n"], st["r0"], st["g"], st["n_str"]
                outb = st["outb"]
                if n < 0:     # merged tail
                    for c in range(3):
                        for nn in range(n_img):
                            nc.sync.dma_start(
                                out=out_d[nn, c, r0:r0 + tail_rows, :],
                                in_=outb[g * c + tail_rows * nn:
                                         g * c + tail_rows * (nn + 1), 0, :])
                    return
                for c in range(3):
                    nc.sync.dma_start(
                        out=out_d[n, c, r0:r0 + g * n_str, :]
                        .rearrange("(s j) w -> j s w", j=g),
                        in_=outb[g * c:g * c + g, :, :])

            # Software-pipelined emission with fine-grained interleave:
            # supertile k's out groups are spread between k+1's input
            # stripes, so out-Ln/Exp(k) isn't queued behind all of k+1's
            # input ACT ops and OSEL(k) fills vector waits on in-MM(k+1).
            st0 = input_dma(0, 0, GB, N_STRIPES, None)
            wm = load_weights(wmain_dram, "m")
            wt = load_weights(wtail_dram, "t") if wtail else None
            st0["wts"] = wm

            stages = []
            for n in range(n_img):
                for gidx in range(n_sup):
                    stages.append((n, gidx * SUP_ROWS, GB, N_STRIPES, wm))
            if tail_rows:
                stages.append((-1, n_sup * SUP_ROWS,
                               n_img * tail_rows, 1, wt))
            prev = None
            for kidx, sgargs in enumerate(stages):
                st = st0 if kidx == 0 else input_dma(*sgargs)
                groups = ([(prev, s, hch) for s in range(prev["n_str"])
                           for hch in range(n_chunk)] if prev else [])
                if prev is not None:
                    prev["outb"] = xio.tile([prev["cp"], prev["n_str"], wid],
                                            F32, tag="outb", name="outb")
                for s in range(st["n_str"]):
                    input_stripe(st, s)
                for grp in groups:
                    out_group(*grp)
                if prev is not None:
                    out_dma(prev)
                mm_chain(st)
                prev = st
            prev["outb"] = xio.tile([prev["cp"], prev["n_str"], wid],
                                    F32, tag="outb", name="outb")
            for s in range(prev["n_str"]):
                for hch in range(n_chunk):
                    out_group(prev, s, hch)
            out_dma(prev)

    nc.compile()
    return nc


_CACHE = {}


def _get_program():
    if "nc" not in _CACHE:
        _CACHE["nc"] = build_program()
    return _CACHE["nc"]


def kernel(im):
    im = np.ascontiguousarray(np.asarray(im), dtype=np.float32)
    assert im.shape == (N_CORES * IMG_PER_CORE, 3, H, WID), im.shape
    nc = _get_program()
    in_maps = [{"im": im[IMG_PER_CORE * i:IMG_PER_CORE * (i + 1)]}
               for i in range(N_CORES)]
    res = run_bass_kernel_spmd(nc, in_maps, core_ids=list(range(N_CORES)))
    return np.concatenate([res.results[i]["out"] for i in range(N_CORES)],
                          axis=0).astype(np.float32)



# revision 41
# speedup vs baseline: 1.0122x; 1.0122x over previous
"""ChromaticityShiftSRGB Trainium2 kernel (8 NeuronCores, SPMD batch-sharded).

v3 (566us v1 -> ~516us): software-pipelined emission + chunked psums.
  - emission order per supertile k: DMA-in(k), per-stripe Ln/Exp/S2L(k),
    out-stage(k-1) [4 MMs + Ln/Exp + OSEL per (stripe, 512-chunk) through
    a 2-deep [126,512] pf ring], DMA-out(k-1), in-MMs + pixel chain(k).
    This puts in-Ln/Exp(k) AHEAD of out-Ln/Exp(k-1) in the scalar queue
    (S2L(k) starts ~4us earlier) and OSEL(k-1) fills the vector engine's
    wait on in-MM(k).
  - input psums are [126,512] ring-2 (3 tags x 1 bank x 2 + pf 2 = 8
    banks): in-MM(k, c1) overlaps chain(k, c0), and in-MM(k+1) never
    waits on chain(k).
  - per-stripe input Ln/Exp/S2L (6 ops each) cuts the input head latency.
  - weights inlined as fp16 and DMA'd directly (no staging CAST on DVE).
  - the fp16 hi/lo DATA split and the y/D weight-split passes are load
    bearing: 11-bit-only y/D flips the sign of v'2 for ~11/16.7M pixels
    sitting on the chromaticity pole, where the reference output is a
    genuine 0<->1 step (error 1.0). Numpy-model absmax (1.1e-2) does NOT
    catch this - the flips come from sub-model-precision perturbations
    (ACT table error etc.) on knife-edge pixels.
  Measured dead ends (kept out): GPSIMD pixel offload (+35us, serializes
  the chain), ACT psum->SBUF copies, fp32r matmuls, fp16-cast input
  (1.85e-2), hi-only data stream (pole flips), pass-major in-MM order
  (hammers one psum bank with 9 consecutive accumulating MMs),
  interleaving OSEL(k-1) between S2L stripes (stalls vector on unready
  OSELs), LDWEIGHTS pairing (LDW fully hidden behind MM at steady state).
  NOTE: HW run-to-run variance is up to ~100us (DVFS/throttle state);
  compare configs with min-of-2 runs.

Pipeline per pixel:
  lin = srgb_to_linear(x); y/D/nu = stripe-sel matmuls (split-fp16 y,D)
  L = select(y<=d3, kappa*y, 116*y^(1/3)-16); S = D/L
  t1 = y/(y - (5/117)S); nu2 = nu + (5/13)S
  A = clamp(t1)*nu2; Kq = clamp(t1)*D
  lin2 = G @ (A, Kq, y);  out = clamp01(linear_to_srgb(lin2))
"""
import sys
import os
import numpy as np

for _p in ("/opt/trn_rl_repo", "/root/.axon_site/_ro/trn_rl_repo"):
    if os.path.isdir(_p) and _p not in sys.path:
        sys.path.insert(0, _p)
        break

import ml_dtypes
import contextlib

import concourse.bass as bass
import concourse.bacc as bacc
import concourse.tile as tile
from concourse import mybir
from concourse.bass_utils import run_bass_kernel_spmd
from concourse import dve_ops
from concourse.dve_spec import (
    Spec, Src0, Src1, C0, C1, C2, AluOp, Bin, select, lower, _has_src1,
    Zero, One, maxx, minn,
)
from concourse.dve_uop import DveOpSpec

F32 = mybir.dt.float32
BF16 = mybir.dt.bfloat16
F16 = mybir.dt.float16
AF = mybir.ActivationFunctionType

N_CORES = 8
IMG_PER_CORE = 2
H = 1024
WID = 1024
GB = 42          # pixel-group block (3*GB=126 partitions)
N_STRIPES = 3    # stripes per supertile
SUP_ROWS = GB * N_STRIPES          # 126 rows per supertile
N_SUP = H // SUP_ROWS              # 8 supertiles
TAIL_ROWS = H - N_SUP * SUP_ROWS   # 16
CHUNK = 512
N_CHUNK = WID // CHUNK             # 2

SPLIT_MODE = os.environ.get("CHR_SPLIT_MODE", "dve")

# Pin Ln+Exp to the shared "natural_log_exp_and_others" ACT table set.
from concourse import hw_specs as _hw_specs

_orig_gat = _hw_specs.get_activation_tables


def _gat_pinned(arch):
    t = _orig_gat(arch)
    for name, funcs in t.items():
        if name != "natural_log_exp_and_others":
            funcs.discard(AF.Ln)
            funcs.discard(AF.Exp)
    return t


bacc.get_activation_tables = _gat_pinned

# ---------------- host-side constants (mirror reference.py) ----------------
_W = np.array([[0.4124564, 0.3575761, 0.1804375],
               [0.2126728, 0.7151522, 0.0721750],
               [0.0193339, 0.1191920, 0.9503041]], dtype=np.float32)
_W_INV = np.linalg.inv(_W.T).astype(np.float32)
_M = _W_INV.T
_w1, _w2, _w3 = _W[0], _W[1], _W[2]
_wD = (_w1 + 15.0 * _w2 + 3.0 * _w3).astype(np.float32)
_N = np.array([[0.25, 0.0, 0.0],
               [0.0, 0.0, 1.0],
               [-1.0 / 12.0, 1.0 / 3.0, -5.0]], dtype=np.float64)
_G = (_M.astype(np.float64) @ _N).astype(np.float32)

DELTA3 = float((6.0 / 29.0) ** 3)
KAPPA = float((29.0 / 3.0) ** 3)
C5_13 = float(5.0 / 13.0)
C5_117 = float(5.0 / 117.0)
LN116 = float(np.log(116.0))
LN1055 = float(np.log(1.055))
MASK_F = 1.9990234375
T1_CLAMP = 1000.0

# ---------------- custom DVE ops ----------------

def _register_op(name, spec, subdim=False):
    for op in dve_ops.OPS:
        if op.name == name:
            return op
    shas = {}
    for ver in ("v3", "v4"):
        uops = lower(spec, ver=ver)
        tmp = DveOpSpec(name=name, opcode=0, uops=uops, rd1_en=_has_src1(spec))
        shas[ver] = tmp.sha(ver)
    op = dve_ops.DveOp(name, spec, subdim, uops_sha=shas)
    dve_ops.OPS.append(op)
    dve_ops.CUSTOM_DVE_SPECS[name] = spec
    dve_ops._SUB_OPCODE_FOR_NAME[name] = (
        dve_ops._CUSTOM_DVE_ROW_BASE + len(dve_ops.OPS) - 1
    )
    assert dve_ops._SUB_OPCODE_FOR_NAME[name] < 0x20
    return op


def _np_mask(c1):
    m = np.ascontiguousarray(np.asarray(c1, np.float32)).view(np.uint32)
    return m if m.ndim == 0 else m.reshape(-1, 1)


def _ref_blend(in0, in1, c0, c2):
    return np.where(in0 <= np.float32(c0),
                    in0.astype(np.float32) * np.float32(c2),
                    in1.astype(np.float32)).astype(np.float32)


def _ref_s2l_hi(in0, in1, c0, c1, c2):
    b = _ref_blend(in0, in1, c0, c2)
    return (b.view(np.uint32) & _np_mask(c1)).view(np.float32)


def _ref_s2l_lo(in0, in1, c0, c1, c2):
    b = _ref_blend(in0, in1, c0, c2)
    hi = (b.view(np.uint32) & _np_mask(c1)).view(np.float32)
    return (b - hi).astype(np.float32)


def _ref_lsel(in0, in1, c0, c1, c2):
    return np.where(in0 <= np.float32(c0),
                    in0.astype(np.float32) * np.float32(c1),
                    in1.astype(np.float32) - np.float32(c2)).astype(np.float32)


def _ref_blend3(in0, in1, c0, c1, c2):
    return _ref_blend(in0, in1, c0, c2)


def _ref_osel(in0, in1, c0, c1, c2):
    v = np.where(in0 <= np.float32(c0),
                 in0.astype(np.float32) * np.float32(c1),
                 in1.astype(np.float32) - np.float32(c2)).astype(np.float32)
    return np.clip(v, np.float32(0.0), np.float32(1.0)).astype(np.float32)


_blend = select(Src0 <= C0, Src0 * C2, Src1)
OP_S2L_HI = _register_op(
    "ANT_CHR_S2L_HI",
    Spec(body=Bin(AluOp.BITWISE_AND, _blend, C1), reference=_ref_s2l_hi))
OP_S2L_LO = _register_op(
    "ANT_CHR_S2L_LO",
    Spec(body=_blend - Bin(AluOp.BITWISE_AND, _blend, C1),
         reference=_ref_s2l_lo))
OP_BLEND = _register_op(
    "ANT_CHR_BLEND",
    Spec(body=select(Src0 <= C0, Src0 * C2, Src1), reference=_ref_blend3))
OP_LSEL = _register_op(
    "ANT_CHR_LSEL",
    Spec(body=select(Src0 <= C0, Src0 * C1, Src1 - C2), reference=_ref_lsel))
# clamp01 folded into the branches; zero synthesized as t-t to stay
# within the DVE's 6 carry lanes (Src0, Src1, C0, C1, C2, One).
_ot = Src0 * C1
_olow = maxx(_ot, Bin(AluOp.SUBTRACT, _ot, _ot))
_ohi = minn(Src1 - C2, One)
OP_OSEL = _register_op(
    "ANT_CHR_OSEL",
    Spec(body=select(Src0 <= C0, _olow, _ohi), reference=_ref_osel))

OP_CLAMP_MUL = _register_op(
    "ANT_CHR_CLAMP_MUL",
    Spec(body=minn(maxx(Src0, C1), C0) * Src1,
         reference=lambda in0, in1, c0, c1, c2:
         (np.clip(in0.astype(np.float32), np.float32(c1), np.float32(c0))
          * in1.astype(np.float32)).astype(np.float32)))

# t1c = clamp(in0*in1, [c1, c0]) — fuses t1 = y*rdv with the +-1000 clamp.
OP_TCLAMP = _register_op(
    "ANT_CHR_TCLAMP",
    Spec(body=minn(maxx(Src0 * Src1, C1), C0),
         reference=lambda in0, in1, c0, c1, c2:
         np.clip(in0.astype(np.float32) * in1.astype(np.float32),
                 np.float32(c1), np.float32(c0)).astype(np.float32)))


# ---------------- weight construction ----------------

def _fp16_grid(a):
    return np.asarray(a, np.float32).astype(np.float16).astype(np.float32)


def _build_weights(g, n_str):
    """Weight matrices for block size g with n_str stripes.
    cp = 3g channel partitions; pp = g*n_str pixel partitions."""
    cp, pp = 3 * g, g * n_str
    out = {}

    def sel_combo(w):
        mats = []
        for s in range(n_str):
            m = np.zeros((cp, pp), np.float32)
            for c in range(3):
                for j in range(g):
                    m[g * c + j, g * s + j] = w[c]
            mats.append(m)
        return mats

    def split(mats):
        his, los = [], []
        for m in mats:
            hi = _fp16_grid(m)
            lo = _fp16_grid(m - hi)
            his.append(hi)
            los.append(lo)
        return his, los

    out["y_hi"], out["y_lo"] = split(sel_combo(_w2))
    out["d_hi"], out["d_lo"] = split(sel_combo(_wD))
    out["p4"] = [_fp16_grid(m) for m in sel_combo(4.0 * _w1)]

    def sel_back(col):
        mats = []
        for s in range(n_str):
            m = np.zeros((pp, cp), np.float32)
            for c2 in range(3):
                for j in range(g):
                    m[g * s + j, g * c2 + j] = _G[c2, col]
            mats.append(m)
        return mats

    out["ga"] = [_fp16_grid(m) for m in sel_back(0)]
    out["gk"] = [_fp16_grid(m) for m in sel_back(1)]
    gy = np.zeros((cp, cp), np.float32)
    for c in range(3):
        for c2 in range(3):
            for j in range(g):
                gy[g * c + j, g * c2 + j] = _w2[c] * _G[c2, 2]
    out["gy"] = [_fp16_grid(gy)]
    return out


# ---------------- program builder ----------------

def build_program(n_img=IMG_PER_CORE, h=H, wid=WID):
    n_sup = h // SUP_ROWS
    tail_rows = h - n_sup * SUP_ROWS
    n_chunk = wid // CHUNK

    nc = bacc.Bacc("TRN2", target_bir_lowering=False, debug=False)
    im_d = nc.declare_dram_parameter("im", [n_img, 3, h, wid], F32, isOutput=False)
    out_d = nc.declare_dram_parameter("out", [n_img, 3, h, wid], F32, isOutput=True)

    wmain = _build_weights(GB, N_STRIPES)
    # Merged tail: both images' leftover rows in ONE stage (g = n_img*16
    # = 32 rows, cp = 96). DVE/ACT op cost is column-bound, so two
    # 16-row tails cost twice what one 32-row tail does.
    wtail = _build_weights(n_img * tail_rows, 1) if tail_rows else None
    assert 3 * n_img * tail_rows <= 128

    def to_dram(wdict, pfx):
        d = {}
        for k, lst in wdict.items():
            for s, m in enumerate(lst):
                d[(k, s)] = nc.inline_tensor(
                    np.ascontiguousarray(m.astype(np.float16)),
                    name=f"w_{pfx}_{k}_{s}")
        return d

    wmain_dram = to_dram(wmain, "m")
    wtail_dram = to_dram(wtail, "t") if wtail else None

    with tile.TileContext(nc) as tc:
        with contextlib.ExitStack() as ctx:
            wp = ctx.enter_context(tc.tile_pool(name="wp", bufs=1))
            xio = ctx.enter_context(tc.tile_pool(name="xio", bufs=3))
            chan = ctx.enter_context(tc.tile_pool(name="chan", bufs=3))
            pix = ctx.enter_context(tc.tile_pool(name="pix", bufs=3))
            ost = ctx.enter_context(tc.tile_pool(name="ost", bufs=2))
            pmm = ctx.enter_context(tc.tile_pool(name="pmm", bufs=2, space="PSUM"))
            pf = ctx.enter_context(tc.tile_pool(name="pf", bufs=2, space="PSUM"))

            # ---- load weights / consts ----
            def load_weights(drams, pfx):
                tiles = {}
                for key, dh in drams.items():
                    shp = list(dh.shape)
                    t = wp.tile(shp, F16, tag=f"w_{pfx}_{key[0]}_{key[1]}")
                    nc.sync.dma_start(out=t[:], in_=dh[:])
                    tiles[key] = t
                return tiles

            # (load_weights is called after the first input DMA below so
            # the first supertile's image rows lead the sync queue)

            def bias(v, name):
                t = wp.tile([128, 1], F32, tag=f"bias_{name}")
                nc.vector.memset(t[:], float(v))
                return t

            b_s2l = bias(0.055 / 1.055, "s2l")
            b_ln116 = bias(LN116, "ln116")
            b_ln1055 = bias(LN1055, "ln1055")
            b_zero = bias(0.0, "zero")

            def input_dma(n, r0, g, n_str, wts):
                """DMA-in only; returns supertile context."""
                cp = 3 * g          # channel-block partitions
                pp = g * n_str      # pixel-block partitions

                x = xio.tile([cp, n_str, wid], F32, tag="x", bufs=4)
                if n < 0:     # merged tail: g rows = n_img * tail_rows
                    for c in range(3):
                        for nn in range(n_img):
                            nc.sync.dma_start(
                                out=x[g * c + tail_rows * nn:
                                      g * c + tail_rows * (nn + 1), 0, :],
                                in_=im_d[nn, c, r0:r0 + tail_rows, :])
                else:
                    for c in range(3):
                        nc.sync.dma_start(
                            out=x[g * c:g * c + g, :, :],
                            in_=im_d[n, c, r0:r0 + g * n_str, :]
                            .rearrange("(s j) w -> j s w", j=g))

                q = chan.tile([cp, n_str, wid], F32, tag="qq", bufs=2)
                hi = chan.tile([cp, n_str, wid], F16, tag="hi")
                lo = chan.tile([cp, n_str, wid], F16, tag="lo")
                return dict(n=n, r0=r0, g=g, n_str=n_str, wts=wts,
                            cp=cp, pp=pp, x=x, q=q, hi=hi, lo=lo)

            def input_stripe(st, s):
                """Ln/Exp + S2L split for one stripe."""
                cp, x, q = st["cp"], st["x"], st["q"]
                hi, lo = st["hi"], st["lo"]
                nc.scalar.activation(q[:, s, :], x[:, s, :], AF.Ln,
                                     bias=b_s2l[0:cp],
                                     scale=float(1 / 1.055))
                nc.scalar.activation(q[:, s, :], q[:, s, :], AF.Exp,
                                     bias=b_zero[0:cp], scale=2.4)
                nc.vector._custom_dve(OP_S2L_HI, out=hi[:, s, :],
                                      in0=x[:, s, :],
                                      in1=q[:, s, :],
                                      s0=0.04045, s1=MASK_F,
                                      imm2=float(1 / 12.92))
                nc.vector._custom_dve(OP_S2L_LO, out=lo[:, s, :],
                                      in0=x[:, s, :],
                                      in1=q[:, s, :],
                                      s0=0.04045, s1=MASK_F,
                                      imm2=float(1 / 12.92))

            def mm_chain(st):
                """Input matmuls + per-chunk pixel chain -> A/Kq (fp16)."""
                pp, n_str, wts = st["pp"], st["n_str"], st["wts"]
                hi, lo = st["hi"], st["lo"]

                A_full = ost.tile([pp, wid], F16, tag="Afull")
                Kq_full = ost.tile([pp, wid], F16, tag="Kqfull")
                for hch in range(n_chunk):
                    cs = slice(CHUNK * hch, CHUNK * (hch + 1))
                    psum_y = pmm.tile([pp, CHUNK], F32, tag="psy")
                    psum_d = pmm.tile([pp, CHUNK], F32, tag="psd")
                    psum_nu = pmm.tile([pp, CHUNK], F32, tag="psnu")
                    for s in range(n_str):
                        first = s == 0
                        last = s == n_str - 1
                        nc.tensor.matmul(psum_y[:], lhsT=wts[("y_hi", s)][:],
                                         rhs=hi[:, s, cs], start=first, stop=False)
                        nc.tensor.matmul(psum_y[:], lhsT=wts[("y_hi", s)][:],
                                         rhs=lo[:, s, cs], start=False, stop=False)
                        nc.tensor.matmul(psum_y[:], lhsT=wts[("y_lo", s)][:],
                                         rhs=hi[:, s, cs], start=False, stop=last)
                        nc.tensor.matmul(psum_d[:], lhsT=wts[("d_hi", s)][:],
                                         rhs=hi[:, s, cs], start=first, stop=False)
                        nc.tensor.matmul(psum_d[:], lhsT=wts[("d_hi", s)][:],
                                         rhs=lo[:, s, cs], start=False, stop=False)
                        nc.tensor.matmul(psum_d[:], lhsT=wts[("d_lo", s)][:],
                                         rhs=hi[:, s, cs], start=False, stop=last)
                        nc.tensor.matmul(psum_nu[:], lhsT=wts[("p4", s)][:],
                                         rhs=hi[:, s, cs], start=first, stop=last)

                    lny = pix.tile([pp, CHUNK], F32, tag="lnyL")
                    nc.scalar.activation(lny[:], psum_y[:], AF.Ln,
                                         bias=b_zero[0:pp], scale=1.0)
                    e3 = pix.tile([pp, CHUNK], F32, tag="e3rL")
                    nc.scalar.activation(e3[:], lny[:], AF.Exp,
                                         bias=b_ln116[0:pp], scale=float(1 / 3))
                    L = pix.tile([pp, CHUNK], F32, tag="lnyL")
                    nc.vector._custom_dve(OP_LSEL, out=L[:], in0=psum_y[:],
                                          in1=e3[:],
                                          s0=DELTA3, s1=KAPPA, imm2=16.0)
                    rL = pix.tile([pp, CHUNK], F32, tag="e3rL")
                    nc.vector.reciprocal_approx_fast(out=rL[:], in_=L[:])
                    S = pix.tile([pp, CHUNK], F32, tag="SKq")
                    nc.vector.tensor_mul(S[:], rL[:], psum_d[:])
                    dv = pix.tile([pp, CHUNK], F32, tag="dvnu")
                    nc.vector.scalar_tensor_tensor(dv[:], in0=S[:],
                                                   scalar=-C5_117, in1=psum_y[:],
                                                   op0=mybir.AluOpType.mult,
                                                   op1=mybir.AluOpType.add)
                    rdv = pix.tile([pp, CHUNK], F32, tag="rdvt1c")
                    nc.vector.reciprocal_approx_fast(out=rdv[:], in_=dv[:])
                    t1 = pix.tile([pp, CHUNK], F32, tag="t1A")
                    nc.vector.tensor_mul(t1[:], rdv[:], psum_y[:])
                    nu2 = pix.tile([pp, CHUNK], F32, tag="dvnu")
                    nc.vector.scalar_tensor_tensor(nu2[:], in0=S[:],
                                                   scalar=C5_13, in1=psum_nu[:],
                                                   op0=mybir.AluOpType.mult,
                                                   op1=mybir.AluOpType.add)
                    nc.vector._custom_dve(OP_CLAMP_MUL, out=A_full[:, cs],
                                          in0=t1[:],
                                          in1=nu2[:], s0=T1_CLAMP, s1=-T1_CLAMP)
                    nc.vector._custom_dve(OP_CLAMP_MUL, out=Kq_full[:, cs],
                                          in0=t1[:],
                                          in1=psum_d[:], s0=T1_CLAMP,
                                          s1=-T1_CLAMP)
                st["A"], st["Kq"] = A_full, Kq_full

            def out_group(st, s, hch):
                """One (stripe, chunk) out group: 4 MMs + Ln/Exp + OSEL."""
                cp, wts = st["cp"], st["wts"]
                hi, lo = st["hi"], st["lo"]
                A_full, Kq_full = st["A"], st["Kq"]
                outb = st["outb"]
                cs = slice(CHUNK * hch, CHUNK * (hch + 1))
                psum_f = pf.tile([cp, CHUNK], F32, tag="psf")
                nc.tensor.matmul(psum_f[:], lhsT=wts[("ga", s)][:],
                                 rhs=A_full[:, cs], start=True, stop=False)
                nc.tensor.matmul(psum_f[:], lhsT=wts[("gk", s)][:],
                                 rhs=Kq_full[:, cs], start=False, stop=False)
                nc.tensor.matmul(psum_f[:], lhsT=wts[("gy", 0)][:],
                                 rhs=hi[:, s, cs], start=False, stop=False)
                nc.tensor.matmul(psum_f[:], lhsT=wts[("gy", 0)][:],
                                 rhs=lo[:, s, cs], start=False, stop=True)

                q2 = ost.tile([cp, CHUNK], F32, tag="q2", bufs=3)
                nc.scalar.activation(q2[:], psum_f[:], AF.Ln,
                                     bias=b_zero[0:cp], scale=1.0)
                E = q2
                nc.scalar.activation(E[:], q2[:], AF.Exp,
                                     bias=b_ln1055[0:cp],
                                     scale=float(1 / 2.4))
                nc.vector._custom_dve(OP_OSEL, out=outb[:, s, cs],
                                      in0=psum_f[:],
                                      in1=E[:], s0=0.0031308, s1=12.92,
                                      imm2=0.055)

            def out_dma(st):
                n, r0, g, n_str = st["n"], st["r0"], st["g"], st["n_str"]
                outb = st["outb"]
                if n < 0:     # merged tail
                    for c in range(3):
                        for nn in range(n_img):
                            nc.sync.dma_start(
                                out=out_d[nn, c, r0:r0 + tail_rows, :],
                                in_=outb[g * c + tail_rows * nn:
                                         g * c + tail_rows * (nn + 1), 0, :])
                    return
                for c in range(3):
                    nc.sync.dma_start(
                        out=out_d[n, c, r0:r0 + g * n_str, :]
                        .rearrange("(s j) w -> j s w", j=g),
                        in_=outb[g * c:g * c + g, :, :])

            # Software-pipelined emission with fine-grained interleave:
            # supertile k's out groups are spread between k+1's input
            # stripes, so out-Ln/Exp(k) isn't queued behind all of k+1's
            # input ACT ops and OSEL(k) fills vector waits on in-MM(k+1).
            st0 = input_dma(0, 0, GB, N_STRIPES, None)
            wm = load_weights(wmain_dram, "m")
            wt = load_weights(wtail_dram, "t") if wtail else None
            st0["wts"] = wm

            stages = []
            for n in range(n_img):
                for gidx in range(n_sup):
                    stages.append((n, gidx * SUP_ROWS, GB, N_STRIPES, wm))
            if tail_rows:
                stages.append((-1, n_sup * SUP_ROWS,
                               n_img * tail_rows, 1, wt))
            prev = None
            for kidx, sgargs in enumerate(stages):
                st = st0 if kidx == 0 else input_dma(*sgargs)
                groups = ([(prev, s, hch) for s in range(prev["n_str"])
                           for hch in range(n_chunk)] if prev else [])
                if prev is not None:
                    prev["outb"] = xio.tile([prev["cp"], prev["n_str"], wid],
                                            F32, tag="outb", name="outb")
                for s in range(st["n_str"]):
                    input_stripe(st, s)
                for grp in groups:
                    out_group(*grp)
                if prev is not None:
                    out_dma(prev)
                mm_chain(st)
                prev = st
            prev["outb"] = xio.tile([prev["cp"], prev["n_str"], wid],
                                    F32, tag="outb", name="outb")
            for s in range(prev["n_str"]):
                for hch in range(n_chunk):
                    out_group(prev, s, hch)
            out_dma(prev)

    nc.compile()
    return nc


_CACHE = {}


def _get_program():
    if "nc" not in _CACHE:
        _CACHE["nc"] = build_program()
    return _CACHE["nc"]


def kernel(im):
    im = np.ascontiguousarray(np.asarray(im), dtype=np.float32)
    assert im.shape == (N_CORES * IMG_PER_CORE, 3, H, WID), im.shape
    nc = _get_program()
    in_maps = [{"im": im[IMG_PER_CORE * i:IMG_PER_CORE * (i + 1)]}
               for i in range(N_CORES)]
    res = run_bass_kernel_spmd(nc, in_maps, core_ids=list(range(N_CORES)))
    return np.concatenate([res.results[i]["out"] for i in range(N_CORES)],
                          axis=0).astype(np.float32)



# revision 42
# speedup vs baseline: 1.0159x; 1.0036x over previous
"""ChromaticityShiftSRGB Trainium2 kernel (8 NeuronCores, SPMD batch-sharded).

v4 (566us v1 -> ~496us): software-pipelined emission + chunked psums,
merged 2-image tail, hoisted first input DMA, q2 ring-3.
  - emission order per supertile k: DMA-in(k), per-stripe Ln/Exp/S2L(k),
    out-stage(k-1) [4 MMs + Ln/Exp + OSEL per (stripe, 512-chunk) through
    a 2-deep [126,512] pf ring], DMA-out(k-1), in-MMs + pixel chain(k).
    This puts in-Ln/Exp(k) AHEAD of out-Ln/Exp(k-1) in the scalar queue
    (S2L(k) starts ~4us earlier) and OSEL(k-1) fills the vector engine's
    wait on in-MM(k).
  - input psums are [126,512] ring-2 (3 tags x 1 bank x 2 + pf 2 = 8
    banks): in-MM(k, c1) overlaps chain(k, c0), and in-MM(k+1) never
    waits on chain(k).
  - per-stripe input Ln/Exp/S2L (6 ops each) cuts the input head latency.
  - weights inlined as fp16 and DMA'd directly (no staging CAST on DVE).
  - the fp16 hi/lo DATA split and the y/D weight-split passes are load
    bearing: 11-bit-only y/D flips the sign of v'2 for ~11/16.7M pixels
    sitting on the chromaticity pole, where the reference output is a
    genuine 0<->1 step (error 1.0). Numpy-model absmax (1.1e-2) does NOT
    catch this - the flips come from sub-model-precision perturbations
    (ACT table error etc.) on knife-edge pixels.
  Measured dead ends (kept out): GPSIMD pixel offload (+35us, serializes
  the chain), ACT psum->SBUF copies, fp32r matmuls, fp16-cast input
  (1.85e-2), hi-only data stream (pole flips), pass-major in-MM order
  (hammers one psum bank with 9 consecutive accumulating MMs),
  interleaving OSEL(k-1) between S2L stripes (stalls vector on unready
  OSELs), LDWEIGHTS pairing (LDW fully hidden behind MM at steady state).
  v4 adds: ONE merged 32-row tail for both images (g=32, cp=96; DVE/ACT
  op cost is column-bound so two 16-row tails cost double), first input
  DMA emitted before the 30 weight DMAs, out-path q2 at ring-3 so
  out-Ln/Exp runs a group ahead of OSEL. pf ring-3 (funded by psum_nu
  ring-1) regressed: the c1 nu-matmul stalls on nu2(c0) mid-burst.
  NOTE: HW run-to-run variance is up to ~100us (DVFS/throttle state);
  compare configs with min-of-2 runs.

Pipeline per pixel:
  lin = srgb_to_linear(x); y/D/nu = stripe-sel matmuls (split-fp16 y,D)
  L = select(y<=d3, kappa*y, 116*y^(1/3)-16); S = D/L
  t1 = y/(y - (5/117)S); nu2 = nu + (5/13)S
  A = clamp(t1)*nu2; Kq = clamp(t1)*D
  lin2 = G @ (A, Kq, y);  out = clamp01(linear_to_srgb(lin2))
"""
import sys
import os
import numpy as np

for _p in ("/opt/trn_rl_repo", "/root/.axon_site/_ro/trn_rl_repo"):
    if os.path.isdir(_p) and _p not in sys.path:
        sys.path.insert(0, _p)
        break

import ml_dtypes
import contextlib

import concourse.bass as bass
import concourse.bacc as bacc
import concourse.tile as tile
from concourse import mybir
from concourse.bass_utils import run_bass_kernel_spmd
from concourse import dve_ops
from concourse.dve_spec import (
    Spec, Src0, Src1, C0, C1, C2, AluOp, Bin, select, lower, _has_src1,
    Zero, One, maxx, minn,
)
from concourse.dve_uop import DveOpSpec

F32 = mybir.dt.float32
BF16 = mybir.dt.bfloat16
F16 = mybir.dt.float16
AF = mybir.ActivationFunctionType

N_CORES = 8
IMG_PER_CORE = 2
H = 1024
WID = 1024
GB = 42          # pixel-group block (3*GB=126 partitions)
N_STRIPES = 3    # stripes per supertile
SUP_ROWS = GB * N_STRIPES          # 126 rows per supertile
N_SUP = H // SUP_ROWS              # 8 supertiles
TAIL_ROWS = H - N_SUP * SUP_ROWS   # 16
CHUNK = 512
N_CHUNK = WID // CHUNK             # 2

SPLIT_MODE = os.environ.get("CHR_SPLIT_MODE", "dve")

# Pin Ln+Exp to the shared "natural_log_exp_and_others" ACT table set.
from concourse import hw_specs as _hw_specs

_orig_gat = _hw_specs.get_activation_tables


def _gat_pinned(arch):
    t = _orig_gat(arch)
    for name, funcs in t.items():
        if name != "natural_log_exp_and_others":
            funcs.discard(AF.Ln)
            funcs.discard(AF.Exp)
    return t


bacc.get_activation_tables = _gat_pinned

# ---------------- host-side constants (mirror reference.py) ----------------
_W = np.array([[0.4124564, 0.3575761, 0.1804375],
               [0.2126728, 0.7151522, 0.0721750],
               [0.0193339, 0.1191920, 0.9503041]], dtype=np.float32)
_W_INV = np.linalg.inv(_W.T).astype(np.float32)
_M = _W_INV.T
_w1, _w2, _w3 = _W[0], _W[1], _W[2]
_wD = (_w1 + 15.0 * _w2 + 3.0 * _w3).astype(np.float32)
_N = np.array([[0.25, 0.0, 0.0],
               [0.0, 0.0, 1.0],
               [-1.0 / 12.0, 1.0 / 3.0, -5.0]], dtype=np.float64)
_G = (_M.astype(np.float64) @ _N).astype(np.float32)

DELTA3 = float((6.0 / 29.0) ** 3)
KAPPA = float((29.0 / 3.0) ** 3)
C5_13 = float(5.0 / 13.0)
C5_117 = float(5.0 / 117.0)
LN116 = float(np.log(116.0))
LN1055 = float(np.log(1.055))
MASK_F = 1.9990234375
T1_CLAMP = 1000.0

# ---------------- custom DVE ops ----------------

def _register_op(name, spec, subdim=False):
    for op in dve_ops.OPS:
        if op.name == name:
            return op
    shas = {}
    for ver in ("v3", "v4"):
        uops = lower(spec, ver=ver)
        tmp = DveOpSpec(name=name, opcode=0, uops=uops, rd1_en=_has_src1(spec))
        shas[ver] = tmp.sha(ver)
    op = dve_ops.DveOp(name, spec, subdim, uops_sha=shas)
    dve_ops.OPS.append(op)
    dve_ops.CUSTOM_DVE_SPECS[name] = spec
    dve_ops._SUB_OPCODE_FOR_NAME[name] = (
        dve_ops._CUSTOM_DVE_ROW_BASE + len(dve_ops.OPS) - 1
    )
    assert dve_ops._SUB_OPCODE_FOR_NAME[name] < 0x20
    return op


def _np_mask(c1):
    m = np.ascontiguousarray(np.asarray(c1, np.float32)).view(np.uint32)
    return m if m.ndim == 0 else m.reshape(-1, 1)


def _ref_blend(in0, in1, c0, c2):
    return np.where(in0 <= np.float32(c0),
                    in0.astype(np.float32) * np.float32(c2),
                    in1.astype(np.float32)).astype(np.float32)


def _ref_s2l_hi(in0, in1, c0, c1, c2):
    b = _ref_blend(in0, in1, c0, c2)
    return (b.view(np.uint32) & _np_mask(c1)).view(np.float32)


def _ref_s2l_lo(in0, in1, c0, c1, c2):
    b = _ref_blend(in0, in1, c0, c2)
    hi = (b.view(np.uint32) & _np_mask(c1)).view(np.float32)
    return (b - hi).astype(np.float32)


def _ref_lsel(in0, in1, c0, c1, c2):
    return np.where(in0 <= np.float32(c0),
                    in0.astype(np.float32) * np.float32(c1),
                    in1.astype(np.float32) - np.float32(c2)).astype(np.float32)


def _ref_blend3(in0, in1, c0, c1, c2):
    return _ref_blend(in0, in1, c0, c2)


def _ref_osel(in0, in1, c0, c1, c2):
    v = np.where(in0 <= np.float32(c0),
                 in0.astype(np.float32) * np.float32(c1),
                 in1.astype(np.float32) - np.float32(c2)).astype(np.float32)
    return np.clip(v, np.float32(0.0), np.float32(1.0)).astype(np.float32)


_blend = select(Src0 <= C0, Src0 * C2, Src1)
OP_S2L_HI = _register_op(
    "ANT_CHR_S2L_HI",
    Spec(body=Bin(AluOp.BITWISE_AND, _blend, C1), reference=_ref_s2l_hi))
OP_S2L_LO = _register_op(
    "ANT_CHR_S2L_LO",
    Spec(body=_blend - Bin(AluOp.BITWISE_AND, _blend, C1),
         reference=_ref_s2l_lo))
OP_BLEND = _register_op(
    "ANT_CHR_BLEND",
    Spec(body=select(Src0 <= C0, Src0 * C2, Src1), reference=_ref_blend3))
OP_LSEL = _register_op(
    "ANT_CHR_LSEL",
    Spec(body=select(Src0 <= C0, Src0 * C1, Src1 - C2), reference=_ref_lsel))
# clamp01 folded into the branches; zero synthesized as t-t to stay
# within the DVE's 6 carry lanes (Src0, Src1, C0, C1, C2, One).
_ot = Src0 * C1
_olow = maxx(_ot, Bin(AluOp.SUBTRACT, _ot, _ot))
_ohi = minn(Src1 - C2, One)
OP_OSEL = _register_op(
    "ANT_CHR_OSEL",
    Spec(body=select(Src0 <= C0, _olow, _ohi), reference=_ref_osel))

OP_CLAMP_MUL = _register_op(
    "ANT_CHR_CLAMP_MUL",
    Spec(body=minn(maxx(Src0, C1), C0) * Src1,
         reference=lambda in0, in1, c0, c1, c2:
         (np.clip(in0.astype(np.float32), np.float32(c1), np.float32(c0))
          * in1.astype(np.float32)).astype(np.float32)))

# t1c = clamp(in0*in1, [c1, c0]) — fuses t1 = y*rdv with the +-1000 clamp.
OP_TCLAMP = _register_op(
    "ANT_CHR_TCLAMP",
    Spec(body=minn(maxx(Src0 * Src1, C1), C0),
         reference=lambda in0, in1, c0, c1, c2:
         np.clip(in0.astype(np.float32) * in1.astype(np.float32),
                 np.float32(c1), np.float32(c0)).astype(np.float32)))


# ---------------- weight construction ----------------

def _fp16_grid(a):
    return np.asarray(a, np.float32).astype(np.float16).astype(np.float32)


def _build_weights(g, n_str):
    """Weight matrices for block size g with n_str stripes.
    cp = 3g channel partitions; pp = g*n_str pixel partitions."""
    cp, pp = 3 * g, g * n_str
    out = {}

    def sel_combo(w):
        mats = []
        for s in range(n_str):
            m = np.zeros((cp, pp), np.float32)
            for c in range(3):
                for j in range(g):
                    m[g * c + j, g * s + j] = w[c]
            mats.append(m)
        return mats

    def split(mats):
        his, los = [], []
        for m in mats:
            hi = _fp16_grid(m)
            lo = _fp16_grid(m - hi)
            his.append(hi)
            los.append(lo)
        return his, los

    out["y_hi"], out["y_lo"] = split(sel_combo(_w2))
    out["d_hi"], out["d_lo"] = split(sel_combo(_wD))
    out["p4"] = [_fp16_grid(m) for m in sel_combo(4.0 * _w1)]

    def sel_back(col):
        mats = []
        for s in range(n_str):
            m = np.zeros((pp, cp), np.float32)
            for c2 in range(3):
                for j in range(g):
                    m[g * s + j, g * c2 + j] = _G[c2, col]
            mats.append(m)
        return mats

    out["ga"] = [_fp16_grid(m) for m in sel_back(0)]
    out["gk"] = [_fp16_grid(m) for m in sel_back(1)]
    gy = np.zeros((cp, cp), np.float32)
    for c in range(3):
        for c2 in range(3):
            for j in range(g):
                gy[g * c + j, g * c2 + j] = _w2[c] * _G[c2, 2]
    out["gy"] = [_fp16_grid(gy)]
    return out


# ---------------- program builder ----------------

def build_program(n_img=IMG_PER_CORE, h=H, wid=WID):
    n_sup = h // SUP_ROWS
    tail_rows = h - n_sup * SUP_ROWS
    n_chunk = wid // CHUNK

    nc = bacc.Bacc("TRN2", target_bir_lowering=False, debug=False)
    im_d = nc.declare_dram_parameter("im", [n_img, 3, h, wid], F32, isOutput=False)
    out_d = nc.declare_dram_parameter("out", [n_img, 3, h, wid], F32, isOutput=True)

    wmain = _build_weights(GB, N_STRIPES)
    # Merged tail: both images' leftover rows in ONE stage (g = n_img*16
    # = 32 rows, cp = 96). DVE/ACT op cost is column-bound, so two
    # 16-row tails cost twice what one 32-row tail does.
    wtail = _build_weights(n_img * tail_rows, 1) if tail_rows else None
    assert 3 * n_img * tail_rows <= 128

    def to_dram(wdict, pfx):
        d = {}
        for k, lst in wdict.items():
            for s, m in enumerate(lst):
                d[(k, s)] = nc.inline_tensor(
                    np.ascontiguousarray(m.astype(np.float16)),
                    name=f"w_{pfx}_{k}_{s}")
        return d

    wmain_dram = to_dram(wmain, "m")
    wtail_dram = to_dram(wtail, "t") if wtail else None

    with tile.TileContext(nc) as tc:
        with contextlib.ExitStack() as ctx:
            wp = ctx.enter_context(tc.tile_pool(name="wp", bufs=1))
            xio = ctx.enter_context(tc.tile_pool(name="xio", bufs=3))
            chan = ctx.enter_context(tc.tile_pool(name="chan", bufs=3))
            pix = ctx.enter_context(tc.tile_pool(name="pix", bufs=3))
            ost = ctx.enter_context(tc.tile_pool(name="ost", bufs=2))
            pmm = ctx.enter_context(tc.tile_pool(name="pmm", bufs=2, space="PSUM"))
            pf = ctx.enter_context(tc.tile_pool(name="pf", bufs=2, space="PSUM"))

            # ---- load weights / consts ----
            def load_weights(drams, pfx):
                tiles = {}
                for key, dh in drams.items():
                    shp = list(dh.shape)
                    t = wp.tile(shp, F16, tag=f"w_{pfx}_{key[0]}_{key[1]}")
                    nc.sync.dma_start(out=t[:], in_=dh[:])
                    tiles[key] = t
                return tiles

            # (load_weights is called after the first input DMA below so
            # the first supertile's image rows lead the sync queue)

            def bias(v, name):
                t = wp.tile([128, 1], F32, tag=f"bias_{name}")
                nc.vector.memset(t[:], float(v))
                return t

            b_s2l = bias(0.055 / 1.055, "s2l")
            b_ln116 = bias(LN116, "ln116")
            b_ln1055 = bias(LN1055, "ln1055")
            b_zero = bias(0.0, "zero")

            def input_dma(n, r0, g, n_str, wts):
                """DMA-in only; returns supertile context."""
                cp = 3 * g          # channel-block partitions
                pp = g * n_str      # pixel-block partitions

                x = xio.tile([cp, n_str, wid], F32, tag="x")
                if n < 0:     # merged tail: g rows = n_img * tail_rows
                    for c in range(3):
                        for nn in range(n_img):
                            nc.sync.dma_start(
                                out=x[g * c + tail_rows * nn:
                                      g * c + tail_rows * (nn + 1), 0, :],
                                in_=im_d[nn, c, r0:r0 + tail_rows, :])
                else:
                    for c in range(3):
                        nc.sync.dma_start(
                            out=x[g * c:g * c + g, :, :],
                            in_=im_d[n, c, r0:r0 + g * n_str, :]
                            .rearrange("(s j) w -> j s w", j=g))

                q = chan.tile([cp, n_str, wid], F32, tag="qq")
                hi = chan.tile([cp, n_str, wid], F16, tag="hi")
                lo = chan.tile([cp, n_str, wid], F16, tag="lo")
                return dict(n=n, r0=r0, g=g, n_str=n_str, wts=wts,
                            cp=cp, pp=pp, x=x, q=q, hi=hi, lo=lo)

            def input_stripe(st, s):
                """Ln/Exp + S2L split for one stripe."""
                cp, x, q = st["cp"], st["x"], st["q"]
                hi, lo = st["hi"], st["lo"]
                nc.scalar.activation(q[:, s, :], x[:, s, :], AF.Ln,
                                     bias=b_s2l[0:cp],
                                     scale=float(1 / 1.055))
                nc.scalar.activation(q[:, s, :], q[:, s, :], AF.Exp,
                                     bias=b_zero[0:cp], scale=2.4)
                nc.vector._custom_dve(OP_S2L_HI, out=hi[:, s, :],
                                      in0=x[:, s, :],
                                      in1=q[:, s, :],
                                      s0=0.04045, s1=MASK_F,
                                      imm2=float(1 / 12.92))
                nc.vector._custom_dve(OP_S2L_LO, out=lo[:, s, :],
                                      in0=x[:, s, :],
                                      in1=q[:, s, :],
                                      s0=0.04045, s1=MASK_F,
                                      imm2=float(1 / 12.92))

            def mm_chain(st):
                """Input matmuls + per-chunk pixel chain -> A/Kq (fp16)."""
                pp, n_str, wts = st["pp"], st["n_str"], st["wts"]
                hi, lo = st["hi"], st["lo"]

                A_full = ost.tile([pp, wid], F16, tag="Afull")
                Kq_full = ost.tile([pp, wid], F16, tag="Kqfull")
                for hch in range(n_chunk):
                    cs = slice(CHUNK * hch, CHUNK * (hch + 1))
                    psum_y = pmm.tile([pp, CHUNK], F32, tag="psy")
                    psum_d = pmm.tile([pp, CHUNK], F32, tag="psd")
                    psum_nu = pmm.tile([pp, CHUNK], F32, tag="psnu")
                    for s in range(n_str):
                        first = s == 0
                        last = s == n_str - 1
                        nc.tensor.matmul(psum_y[:], lhsT=wts[("y_hi", s)][:],
                                         rhs=hi[:, s, cs], start=first, stop=False)
                        nc.tensor.matmul(psum_y[:], lhsT=wts[("y_hi", s)][:],
                                         rhs=lo[:, s, cs], start=False, stop=False)
                        nc.tensor.matmul(psum_y[:], lhsT=wts[("y_lo", s)][:],
                                         rhs=hi[:, s, cs], start=False, stop=last)
                        nc.tensor.matmul(psum_d[:], lhsT=wts[("d_hi", s)][:],
                                         rhs=hi[:, s, cs], start=first, stop=False)
                        nc.tensor.matmul(psum_d[:], lhsT=wts[("d_hi", s)][:],
                                         rhs=lo[:, s, cs], start=False, stop=False)
                        nc.tensor.matmul(psum_d[:], lhsT=wts[("d_lo", s)][:],
                                         rhs=hi[:, s, cs], start=False, stop=last)
                        nc.tensor.matmul(psum_nu[:], lhsT=wts[("p4", s)][:],
                                         rhs=hi[:, s, cs], start=first, stop=last)

                    lny = pix.tile([pp, CHUNK], F32, tag="lnyL")
                    nc.scalar.activation(lny[:], psum_y[:], AF.Ln,
                                         bias=b_zero[0:pp], scale=1.0)
                    e3 = pix.tile([pp, CHUNK], F32, tag="e3rL")
                    nc.scalar.activation(e3[:], lny[:], AF.Exp,
                                         bias=b_ln116[0:pp], scale=float(1 / 3))
                    L = pix.tile([pp, CHUNK], F32, tag="lnyL")
                    nc.vector._custom_dve(OP_LSEL, out=L[:], in0=psum_y[:],
                                          in1=e3[:],
                                          s0=DELTA3, s1=KAPPA, imm2=16.0)
                    rL = pix.tile([pp, CHUNK], F32, tag="e3rL")
                    nc.vector.reciprocal_approx_fast(out=rL[:], in_=L[:])
                    S = pix.tile([pp, CHUNK], F32, tag="SKq")
                    nc.vector.tensor_mul(S[:], rL[:], psum_d[:])
                    dv = pix.tile([pp, CHUNK], F32, tag="dvnu")
                    nc.vector.scalar_tensor_tensor(dv[:], in0=S[:],
                                                   scalar=-C5_117, in1=psum_y[:],
                                                   op0=mybir.AluOpType.mult,
                                                   op1=mybir.AluOpType.add)
                    rdv = pix.tile([pp, CHUNK], F32, tag="rdvt1c")
                    nc.vector.reciprocal_approx_fast(out=rdv[:], in_=dv[:])
                    t1 = pix.tile([pp, CHUNK], F32, tag="t1A")
                    nc.vector.tensor_mul(t1[:], rdv[:], psum_y[:])
                    nu2 = pix.tile([pp, CHUNK], F32, tag="dvnu")
                    nc.vector.scalar_tensor_tensor(nu2[:], in0=S[:],
                                                   scalar=C5_13, in1=psum_nu[:],
                                                   op0=mybir.AluOpType.mult,
                                                   op1=mybir.AluOpType.add)
                    nc.vector._custom_dve(OP_CLAMP_MUL, out=A_full[:, cs],
                                          in0=t1[:],
                                          in1=nu2[:], s0=T1_CLAMP, s1=-T1_CLAMP)
                    nc.vector._custom_dve(OP_CLAMP_MUL, out=Kq_full[:, cs],
                                          in0=t1[:],
                                          in1=psum_d[:], s0=T1_CLAMP,
                                          s1=-T1_CLAMP)
                st["A"], st["Kq"] = A_full, Kq_full

            def out_group(st, s, hch):
                """One (stripe, chunk) out group: 4 MMs + Ln/Exp + OSEL."""
                cp, wts = st["cp"], st["wts"]
                hi, lo = st["hi"], st["lo"]
                A_full, Kq_full = st["A"], st["Kq"]
                outb = st["outb"]
                cs = slice(CHUNK * hch, CHUNK * (hch + 1))
                psum_f = pf.tile([cp, CHUNK], F32, tag="psf")
                nc.tensor.matmul(psum_f[:], lhsT=wts[("ga", s)][:],
                                 rhs=A_full[:, cs], start=True, stop=False)
                nc.tensor.matmul(psum_f[:], lhsT=wts[("gk", s)][:],
                                 rhs=Kq_full[:, cs], start=False, stop=False)
                nc.tensor.matmul(psum_f[:], lhsT=wts[("gy", 0)][:],
                                 rhs=hi[:, s, cs], start=False, stop=False)
                nc.tensor.matmul(psum_f[:], lhsT=wts[("gy", 0)][:],
                                 rhs=lo[:, s, cs], start=False, stop=True)

                q2 = ost.tile([cp, CHUNK], F32, tag="q2", bufs=3)
                nc.scalar.activation(q2[:], psum_f[:], AF.Ln,
                                     bias=b_zero[0:cp], scale=1.0)
                E = q2
                nc.scalar.activation(E[:], q2[:], AF.Exp,
                                     bias=b_ln1055[0:cp],
                                     scale=float(1 / 2.4))
                nc.vector._custom_dve(OP_OSEL, out=outb[:, s, cs],
                                      in0=psum_f[:],
                                      in1=E[:], s0=0.0031308, s1=12.92,
                                      imm2=0.055)

            def out_dma(st):
                n, r0, g, n_str = st["n"], st["r0"], st["g"], st["n_str"]
                outb = st["outb"]
                if n < 0:     # merged tail
                    for c in range(3):
                        for nn in range(n_img):
                            nc.sync.dma_start(
                                out=out_d[nn, c, r0:r0 + tail_rows, :],
                                in_=outb[g * c + tail_rows * nn:
                                         g * c + tail_rows * (nn + 1), 0, :])
                    return
                for c in range(3):
                    nc.sync.dma_start(
                        out=out_d[n, c, r0:r0 + g * n_str, :]
                        .rearrange("(s j) w -> j s w", j=g),
                        in_=outb[g * c:g * c + g, :, :])

            # Software-pipelined emission with fine-grained interleave:
            # supertile k's out groups are spread between k+1's input
            # stripes, so out-Ln/Exp(k) isn't queued behind all of k+1's
            # input ACT ops and OSEL(k) fills vector waits on in-MM(k+1).
            st0 = input_dma(0, 0, GB, N_STRIPES, None)
            wm = load_weights(wmain_dram, "m")
            wt = load_weights(wtail_dram, "t") if wtail else None
            st0["wts"] = wm

            stages = []
            for n in range(n_img):
                for gidx in range(n_sup):
                    stages.append((n, gidx * SUP_ROWS, GB, N_STRIPES, wm))
            if tail_rows:
                stages.append((-1, n_sup * SUP_ROWS,
                               n_img * tail_rows, 1, wt))
            prev = None
            for kidx, sgargs in enumerate(stages):
                st = st0 if kidx == 0 else input_dma(*sgargs)
                groups = ([(prev, s, hch) for s in range(prev["n_str"])
                           for hch in range(n_chunk)] if prev else [])
                if prev is not None:
                    prev["outb"] = xio.tile([prev["cp"], prev["n_str"], wid],
                                            F32, tag="outb", name="outb")
                for s in range(st["n_str"]):
                    input_stripe(st, s)
                for grp in groups:
                    out_group(*grp)
                if prev is not None:
                    out_dma(prev)
                mm_chain(st)
                prev = st
            prev["outb"] = xio.tile([prev["cp"], prev["n_str"], wid],
                                    F32, tag="outb", name="outb")
            for s in range(prev["n_str"]):
                for hch in range(n_chunk):
                    out_group(prev, s, hch)
            out_dma(prev)

    nc.compile()
    return nc


_CACHE = {}


def _get_program():
    if "nc" not in _CACHE:
        _CACHE["nc"] = build_program()
    return _CACHE["nc"]


def kernel(im):
    im = np.ascontiguousarray(np.asarray(im), dtype=np.float32)
    assert im.shape == (N_CORES * IMG_PER_CORE, 3, H, WID), im.shape
    nc = _get_program()
    in_maps = [{"im": im[IMG_PER_CORE * i:IMG_PER_CORE * (i + 1)]}
               for i in range(N_CORES)]
    res = run_bass_kernel_spmd(nc, in_maps, core_ids=list(range(N_CORES)))
    return np.concatenate([res.results[i]["out"] for i in range(N_CORES)],
                          axis=0).astype(np.float32)

